# revision 33
# baseline (speedup 1.0000x reference)
"""Fused multi-head attention (B=4, N=2048, C=1024, H=16, D=64) on 8 NeuronCores.

Sharding: core i handles batch b = i // 2, head-group g = i % 2 (heads
8g..8g+7).  Each core runs an identical Bass/Tile program (SPMD).

v2 schedule: the ScalarE exp stream (256 x [128,1024] activations) is the
binding engine together with PE; the program is organized as one flat loop of
256 attention steps (pair p, 512-token window w, key-chunk m) so that:
  - exp starts ~15us in: only pair-0's Q/K projection gates it (weights are
    host-split so those columns DMA first);
  - all remaining projection work (Q/K of pairs 1-3, all of V) trickles into
    per-step PE slack through a dedicated 1-bank PSUM ring;
  - AV matmuls trail the exp stream by 12 steps (pt ring keeps 16 tiles), so
    V-projection in the first 16 steps never stalls ScalarE;
  - S matmuls for the two heads of a pair use PE row-tiles (0,0)/(64,0)
    (K=64 each) so they can run concurrently on hardware.
Scores are computed transposed (S.T = K.T' @ Q.T per head); exp on ScalarE
(no max subtraction: |S|*0.125 <= ~6); V carries an appended ones column so
softmax denominators fall out of the AV matmul; a PE transpose + per-row
1/den scaling produces the output in natural layout.  All matmuls run in
bf16 with fp32 PSUM accumulation.
"""

from contextlib import ExitStack

import ml_dtypes
import numpy as np

import concourse.bass as bass
import concourse.mybir as mybir
import concourse.tile as tile
from concourse import bacc
from concourse.masks import make_identity

dt = mybir.dt
AF = mybir.ActivationFunctionType
BF16 = dt.bfloat16
F32 = dt.float32

B, N_TOK, C_IN = 4, 2048, 1024
NH = 8            # heads per core
NPAIR = NH // 2   # head pairs
D = 64
KC = C_IN // 128  # contraction k-tiles
MT = N_TOK // 128 # key-token chunks per window loop
NW = N_TOK // 512 # 512-token output windows per pair
VROW = 65         # V columns per head incl. ones column
NSTEP = NPAIR * NW * MT  # 256 flat attention steps
NWIN = NPAIR * NW        # 16 (pair, window) units
# AV trails exp: enough slack early on that trickled V-projection never
# stalls ScalarE, tapering late so the post-loop AV flush tail is short.
# av PSUM ring-1 requires TRAIL[k+1] >= TRAIL[k] - 1.
TRAIL_BY_WIN = [max(10, 23 - max(0, k - 2)) for k in range(NWIN)]


def build_nc(iters: int = 1):
    nc = bacc.Bacc(trn_type="TRN2")
    xT = nc.dram_tensor("xT", [C_IN, N_TOK], BF16, kind="ExternalInput").ap()
    wT1 = nc.dram_tensor("wT1", [C_IN, 256], BF16, kind="ExternalInput").ap()
    wT2 = nc.dram_tensor("wT2", [C_IN, 768], BF16, kind="ExternalInput").ap()
    wTv = nc.dram_tensor("wTv", [C_IN, 512], BF16, kind="ExternalInput").ap()
    qkb = nc.dram_tensor("qkb", [1024], F32, kind="ExternalInput").ap()
    vb = nc.dram_tensor("vb", [512], F32, kind="ExternalInput").ap()
    out = nc.dram_tensor("out", [N_TOK, NH * D], F32, kind="ExternalOutput").ap()

    with tile.TileContext(nc) as tc, ExitStack() as ctx:
        consts = ctx.enter_context(tc.tile_pool(name="consts", bufs=1))
        p_xt = ctx.enter_context(tc.tile_pool(name="p_xt", bufs=1))
        p_w1 = ctx.enter_context(tc.tile_pool(name="p_w1", bufs=1))
        p_w2 = ctx.enter_context(tc.tile_pool(name="p_w2", bufs=1))
        p_wv = ctx.enter_context(tc.tile_pool(name="p_wv", bufs=1))
        p_qk = ctx.enter_context(tc.tile_pool(name="p_qk", bufs=2 * NPAIR))
        p_vp = ctx.enter_context(tc.tile_pool(name="p_vp", bufs=MT))
        p_pt = ctx.enter_context(tc.tile_pool(name="p_pt", bufs=32))
        p_osb = ctx.enter_context(tc.tile_pool(name="p_osb", bufs=4))
        p_eps = ctx.enter_context(tc.tile_pool(name="p_eps", bufs=4))

        identity = consts.tile([128, 128], BF16, name="identity")
        make_identity(nc, identity)
        qkb_sb = consts.tile([128, 8], F32, name="qkb_sb")
        nc.sync.dma_start(out=qkb_sb, in_=qkb.rearrange("(t p) -> p t", p=128))
        vb_bc = consts.tile([128, 512], F32, name="vb_bc")
        nc.sync.dma_start(
            out=vb_bc,
            in_=bass.AP(tensor=vb.tensor, offset=vb.offset, ap=[[0, 128], vb.ap[0]]),
        )
        # preload the exp table set while input DMAs run
        warm = consts.tile([128, 1], F32, name="warm")
        nc.scalar.activation(out=warm, in_=qkb_sb[:, 0:1], func=AF.Exp)

        def body():
            # ---- input DMAs: one multi-dim-AP DMA per tensor (queue/SEQ
            # cost per dma_start is ~650ns; merging is a big head saving),
            # ordered by first use: w1+xtA gate the first S, wv gates the
            # trickled V units (steps 6+), xtB gates kU(0,2/3) (steps 2/4),
            # w2 gates pair-1 units (step 22+). ----
            w1t = p_w1.tile([128, KC * 256], BF16, name="w1", tag="w1")
            nc.sync.dma_start(
                out=w1t.rearrange("p (kc c) -> p kc c", kc=KC),
                in_=wT1.rearrange("(kc p) c -> p kc c", p=128),
            )
            xtt = p_xt.tile([128, KC * N_TOK], BF16, name="xt", tag="xt")
            xt3_sb = xtt.rearrange("p (kc c) -> p kc c", kc=KC)
            xt3_hbm = xT.rearrange("(kc p) c -> p kc c", p=128)
            nc.sync.dma_start(out=xt3_sb[:, :, 0:512], in_=xt3_hbm[:, :, 0:512])
            nc.sync.dma_start(out=xt3_sb[:, :, 512:1024], in_=xt3_hbm[:, :, 512:1024])
            wvt = p_wv.tile([128, KC * 512], BF16, name="wv", tag="wv")
            nc.sync.dma_start(
                out=wvt.rearrange("p (kc c) -> p kc c", kc=KC),
                in_=wTv.rearrange("(kc p) c -> p kc c", p=128),
            )
            nc.sync.dma_start(out=xt3_sb[:, :, 1024:2048], in_=xt3_hbm[:, :, 1024:2048])
            w2t = p_w2.tile([128, KC * 768], BF16, name="w2", tag="w2")
            nc.sync.dma_start(
                out=w2t.rearrange("p (kc c) -> p kc c", kc=KC),
                in_=wT2.rearrange("(kc p) c -> p kc c", p=128),
            )

            def xts(kc, sl):
                return xtt[:, kc * N_TOK + sl.start : kc * N_TOK + sl.stop]

            wv = [wvt[:, kc * 512 : (kc + 1) * 512] for kc in range(KC)]

            # persistent SBUF tiles
            qk = [
                p_qk.tile([128, N_TOK], BF16, name=f"qk{u}", tag="qk")
                for u in range(2 * NPAIR)
            ]
            vp = [
                p_vp.tile([128, NH * VROW], BF16, name=f"vp{tt}", tag="vp")
                for tt in range(MT)
            ]

            def w_slice(u, kc):
                # unit u: 0=q0,1=k0 from wT1; 2..7 = q1,k1,q2,k2,q3,k3 from wT2
                if u < 2:
                    return w1t[:, kc * 256 + u * 128 : kc * 256 + (u + 1) * 128]
                base = kc * 768 + (u - 2) * 128
                return w2t[:, base : base + 128]

            # Trickled projection units are emitted as single-matmul closures
            # drained a few per attention step, so PE bursts between ACT
            # feeds stay short. `state` carries the unit's pj tile.
            def qk_unit_mms(pp_proj, u, tb):
                state = {}

                def mm(kc):
                    if kc == 0:
                        state["pj"] = pp_proj.tile(
                            [128, 512], F32, name=f"pj{u}_{tb}", tag="pj"
                        )
                    nc.tensor.matmul(
                        state["pj"],
                        lhsT=w_slice(u, kc),
                        rhs=xts(kc, slice(tb * 512, (tb + 1) * 512)),
                        start=(kc == 0),
                        stop=(kc == KC - 1),
                    )
                    if kc == KC - 1:
                        nc.vector.tensor_scalar_add(
                            out=qk[u][:, tb * 512 : (tb + 1) * 512],
                            in0=state["pj"],
                            scalar1=qkb_sb[:, u : u + 1],
                        )

                return [lambda kc=kc: mm(kc) for kc in range(KC)]

            def v_unit_mms(pp_proj, tt):
                state = {}

                def mm(kc):
                    if kc == 0:
                        state["pj"] = pp_proj.tile(
                            [128, 512], F32, name=f"pv{tt}", tag="pj"
                        )
                    nc.tensor.matmul(
                        state["pj"],
                        lhsT=xts(kc, slice(tt * 128, (tt + 1) * 128)),
                        rhs=wv[kc],
                        start=(kc == 0),
                        stop=(kc == KC - 1),
                    )
                    if kc == KC - 1:
                        t3 = vp[tt].rearrange("p (h d) -> p h d", h=NH)
                        nc.gpsimd.memset(t3[:, :, 64:65], 1.0)
                        nc.vector.tensor_add(
                            out=t3[:, :, 0:64],
                            in0=state["pj"].rearrange("p (h d) -> p h d", h=NH),
                            in1=vb_bc.rearrange("p (h d) -> p h d", h=NH),
                        )

                return [lambda kc=kc: mm(kc) for kc in range(KC)]

            # trickle schedule: flat step -> list of (kind, args)
            emits = {}

            def emit_at(i, item):
                emits.setdefault(i, []).append(item)

            emit_at(2, ("qk", 1, 2))
            emit_at(4, ("qk", 1, 3))
            for t in range(MT):
                emit_at(6 + 2 * t, ("v", t))  # even steps 6..36; deadline t+TRAIL
            emit_at(9, ("qk", 0, 1))
            emit_at(25, ("qk", 0, 2))
            emit_at(41, ("qk", 0, 3))
            for p in range(1, NPAIR):
                for w in range(1, NW):
                    emit_at(64 * p + 16 * w - 8, ("qk", 2 * p, w))
                emit_at(64 * p - 41, ("qk", 2 * p, 0))
                for tb in range(NW):
                    emit_at(64 * p - 33 + 8 * tb, ("qk", 2 * p + 1, tb))

            with tc.tile_pool(name="pp_s", bufs=2, space="PSUM") as pp_s, \
                 tc.tile_pool(name="pp_av", bufs=1, space="PSUM") as pp_av, \
                 tc.tile_pool(name="pp_proj", bufs=1, space="PSUM") as pp_proj, \
                 tc.tile_pool(name="pp_tr", bufs=1, space="PSUM") as pp_tr:

                def epilogue(k, av_t):
                    p, w = divmod(k, NW)
                    for hh in range(2):
                        h = 2 * p + hh
                        osb = p_osb.tile([VROW, 512], BF16, name=f"osb{k}_{hh}", tag="osb")
                        nc.vector.tensor_copy(out=osb, in_=av_t[hh])
                        ob = p_eps.tile([128, 256], F32, name="ob", tag="ob")
                        for c in range(4):
                            tr = pp_tr.tile([128, VROW], BF16, name="tr", tag="tr")
                            nc.tensor.transpose(
                                tr,
                                in_=osb[:, c * 128 : (c + 1) * 128],
                                identity=identity[0:VROW, 0:VROW],
                            )
                            rc = p_eps.tile([128, 1], F32, name="rc", tag="rc")
                            nc.vector.reciprocal(out=rc, in_=tr[:, 64:65])
                            nc.vector.tensor_scalar_mul(
                                out=ob[:, c * 64 : (c + 1) * 64],
                                in0=tr[:, 0:64],
                                scalar1=rc,
                            )
                        # one DMA for the whole [512-token, 64-dim] block, on
                        # the SP HWDGE queue (idle after the 7 input DMAs;
                        # ACT-queue dispatch would bubble the exp stream)
                        dst = out[w * 512 : (w + 1) * 512, h * 64 : (h + 1) * 64]
                        nc.sync.dma_start(
                            out=dst.rearrange("(c p) d -> p c d", p=128),
                            in_=ob.rearrange("p (c d) -> p c d", c=4),
                        )

                # head units: pair-0 Q window 0 + K for tokens 0:1024
                for u, tb in ((0, 0), (1, 0), (1, 1)):
                    for f in qk_unit_mms(pp_proj, u, tb):
                        f()

                done_step = {}

                pt_ring = [None] * NSTEP
                av_by_win = {}
                # av_emits[i] = AV steps to issue after S/exp of flat step i
                av_emits = {}
                for j in range(NSTEP):
                    k = j // MT
                    av_emits.setdefault(j + TRAIL_BY_WIN[k], []).append(j)

                def av_step(j):
                    p, rem = divmod(j, NW * MT)
                    w, m = divmod(rem, MT)
                    k = p * NW + w
                    if m == 0:
                        av_by_win[k] = [
                            pp_av.tile([VROW, 512], F32, name=f"av{k}_{hh}",
                                       tag=f"av{hh}")
                            for hh in range(2)
                        ]
                    av_t = av_by_win[k]
                    pt = pt_ring[j]
                    for hh in range(2):
                        h = 2 * p + hh
                        nc.tensor.matmul(
                            av_t[hh],
                            lhsT=vp[m][:, h * VROW : (h + 1) * VROW],
                            rhs=pt[:, hh * 512 : (hh + 1) * 512],
                            start=(m == 0),
                            stop=(m == MT - 1),
                        )
                    pt_ring[j] = None
                    if m == MT - 1:
                        epilogue(k, av_t)

                for i in range(NSTEP):
                    p, rem = divmod(i, NW * MT)
                    w, m = divmod(rem, MT)
                    # S + exp for step i
                    s = pp_s.tile([128, 1024], F32, name="s", tag="s")
                    for hh in range(2):
                        rows = slice(hh * 64, (hh + 1) * 64)
                        nc.tensor.matmul(
                            s[:, hh * 512 : (hh + 1) * 512],
                            lhsT=qk[2 * p + 1][rows, m * 128 : (m + 1) * 128],
                            rhs=qk[2 * p][rows, w * 512 : (w + 1) * 512],
                            start=True,
                            stop=True,
                            tile_position=(hh * 64, 0),
                        )
                    pt = p_pt.tile([128, 1024], BF16, name=f"pt{i % 16}", tag="pt")
                    nc.scalar.activation(out=pt, in_=s, func=AF.Exp, scale=0.125)
                    pt_ring[i] = pt
                    # trickled projection work, one whole unit per step.
                    # Tile dependencies follow emission order, so each unit's
                    # final write MUST be emitted before its first consumer —
                    # checked via done_step below.
                    for item in emits.get(i, ()):
                        mms = (v_unit_mms(pp_proj, item[1]) if item[0] == "v"
                               else qk_unit_mms(pp_proj, item[1], item[2]))
                        for f in mms:
                            f()
                        done_step[item] = i
                    # trailing AV
                    for j in av_emits.get(i, ()):
                        av_step(j)
                for i in range(NSTEP, NSTEP + TRAIL_BY_WIN[-1]):
                    for j in av_emits.get(i, ()):
                        av_step(j)
                # every trickled unit's last write emitted before its first
                # consumer's emission step
                for (kind, *args), ds in done_step.items():
                    if kind == "v":
                        first_use = args[0] + TRAIL_BY_WIN[0]
                    else:
                        u, tb = args
                        p, isk = divmod(u, 2)
                        first_use = 64 * p + (4 * tb if isk else 16 * tb)
                    assert ds < first_use, (
                        f"unit {(kind, *args)} finalized at step {ds}, "
                        f"first consumer emitted at step {first_use}"
                    )

        for _ in range(iters):
            body()

    nc.finalize()
    return nc


_NC_CACHE = {}


def _get_nc(iters: int = 1):
    if iters not in _NC_CACHE:
        _NC_CACHE[iters] = build_nc(iters)
    return _NC_CACHE[iters]


def make_in_maps(x, qkv_w, qkv_b):
    bf = ml_dtypes.bfloat16
    in_maps = []
    for core in range(8):
        b, g = core // 2, core % 2
        xTc = np.ascontiguousarray(x[b].T).astype(bf)
        wq = qkv_w[g * 512 : (g + 1) * 512]
        wk = qkv_w[1024 + g * 512 : 1024 + (g + 1) * 512]
        wv = qkv_w[2048 + g * 512 : 2048 + (g + 1) * 512]
        bq = qkv_b[g * 512 : (g + 1) * 512]
        bk = qkv_b[1024 + g * 512 : 1024 + (g + 1) * 512]
        bv = qkv_b[2048 + g * 512 : 2048 + (g + 1) * 512]
        w1 = np.concatenate([wq[0:128], wk[0:128]], axis=0)
        w2 = np.concatenate(
            [arr for f in range(1, 4) for arr in (wq[f * 128 : (f + 1) * 128],
                                                  wk[f * 128 : (f + 1) * 128])],
            axis=0,
        )
        qkbc = np.concatenate(
            [arr for f in range(4) for arr in (bq[f * 128 : (f + 1) * 128],
                                               bk[f * 128 : (f + 1) * 128])]
        )
        in_maps.append(
            {
                "xT": xTc,
                "wT1": np.ascontiguousarray(w1.T).astype(bf),
                "wT2": np.ascontiguousarray(w2.T).astype(bf),
                "wTv": np.ascontiguousarray(wv.T).astype(bf),
                "qkb": np.ascontiguousarray(qkbc).astype(np.float32),
                "vb": np.ascontiguousarray(bv).astype(np.float32),
            }
        )
    return in_maps


_RUNNER_CACHE = {}


def _get_runner(iters: int = 1, n_cores: int = 8):
    """Build the shard_map-wrapped bass_exec executable once and reuse it, so
    repeated kernel() calls don't re-ship the NEFF through the axon tunnel."""
    if iters in _RUNNER_CACHE:
        return _RUNNER_CACHE[iters]
    import jax
    from jax.sharding import Mesh, PartitionSpec
    from jax.experimental.shard_map import shard_map
    from concourse.bass2jax import (
        _bass_exec_p,
        install_neuronx_cc_hook,
        partition_id_tensor,
    )

    nc = _get_nc(iters)
    install_neuronx_cc_hook()
    partition_name = nc.partition_id_tensor.name if nc.partition_id_tensor else None
    in_names, out_names, out_avals, zero_outs = [], [], [], []
    for alloc in nc.m.functions[0].allocations:
        if not isinstance(alloc, mybir.MemoryLocationSet):
            continue
        name = alloc.memorylocations[0].name
        if alloc.kind == "ExternalInput":
            if name != partition_name:
                in_names.append(name)
        elif alloc.kind == "ExternalOutput":
            shape = tuple(alloc.tensor_shape)
            npdt = dt.np(alloc.dtype)
            out_names.append(name)
            out_avals.append(jax.core.ShapedArray(shape, npdt))
            zero_outs.append(np.zeros(shape, npdt))
    n_params = len(in_names)
    all_in_names = list(in_names) + list(out_names)
    if partition_name is not None:
        all_in_names.append(partition_name)

    def _body(*args):
        operands = list(args)
        if partition_name is not None:
            operands.append(partition_id_tensor())
        return tuple(
            _bass_exec_p.bind(
                *operands,
                out_avals=tuple(out_avals),
                in_names=tuple(all_in_names),
                out_names=tuple(out_names),
                lowering_input_output_aliases=(),
                sim_require_finite=True,
                sim_require_nnan=True,
                nc=nc,
            )
        )

    devices = jax.devices()[:n_cores]
    mesh = Mesh(np.asarray(devices), ("core",))
    in_specs = (PartitionSpec("core"),) * (n_params + len(out_names))
    out_specs = (PartitionSpec("core"),) * len(out_names)
    fn = jax.jit(
        shard_map(_body, mesh=mesh, in_specs=in_specs, out_specs=out_specs, check_rep=False)
    )
    zero_concat = [
        np.zeros((n_cores * z.shape[0], *z.shape[1:]), z.dtype) for z in zero_outs
    ]
    _RUNNER_CACHE[iters] = (fn, in_names, zero_concat, mesh)
    return _RUNNER_CACHE[iters]


def kernel(x, qkv_w, qkv_b):
    import jax

    x = np.asarray(x, dtype=np.float32)
    qkv_w = np.asarray(qkv_w, dtype=np.float32)
    qkv_b = np.asarray(qkv_b, dtype=np.float32)
    in_maps = make_in_maps(x, qkv_w, qkv_b)
    fn, in_names, zero_concat, _ = _get_runner(1)
    concat_in = [
        np.concatenate([in_maps[c][name] for c in range(8)], axis=0) for name in in_names
    ]
    outs = fn(*concat_in, *zero_concat)
    out_global = np.asarray(jax.block_until_ready(outs)[0])
    full = np.empty((B, N_TOK, C_IN), dtype=np.float32)
    for core in range(8):
        b, g = core // 2, core % 2
        full[b, :, g * 512 : (g + 1) * 512] = out_global[core * N_TOK : (core + 1) * N_TOK]
    return full
